# revision 7
# baseline (speedup 1.0000x reference)
"""Trainium2 Bass kernel for the HNN pairwise-potential module.

Math: for each batch b and each unordered pair (i<j) of the N=1024 points,
  d = sqrt(||p_i - p_j||^2 + eps^2)
  u(d) = W3·silu(W2ᵀ·silu(d·W1 + b1) + b2) + b3
  U[b] = sum_pairs u(d) / N

u is a smooth scalar function of the single scalar d, so instead of running
the 64-wide MLP per pair (ScalarE-bound, ~425 us/core), the host fits a
degree-8 Chebyshev polynomial p(y) ~= u(D*y) on y in [0,1] (D = upper bound
on d from the actual positions; fit rebuilt per call from the actual MLP
weights, max fit error ~1.4e-3 which is ~5000x inside the 2e-2 tolerance
after summing 524k pairs).

Device strategy (8 cores, 2 per batch; 18 128x128 pair blocks per core:
14 off-diagonal + 4 full diagonal blocks, corrected exactly on the host):
  - PE: per block one K=5 matmul produces y^2 = (|pi-pj|^2+eps^2)/D^2
    directly in PSUM (rows: -2*p_i/D^2, (|p_i|^2+eps^2)/D^2, 1 against
    p_j, 1, |p_j|^2/D^2). float32r, 4 blocks per PSUM bank.
  - ScalarE: one Sqrt activation per 4-block group -> y tile [128, 2304].
  - DVE + Pool: Horner chain z = (z + a_k)*y via scalar_tensor_tensor,
    8 passes, columns split DVE [0:1312) / Pool [1312:2304) to balance
    their 0.96 vs 0.70 cols/ns rates; the final step fuses the per-
    partition reduction via accum_out. a_0 and the diagonal double-count
    are applied exactly on the host (they commute with the sum).
  - Output: [128, 4] partial sums (3 off-diagonal chunks + 1 diagonal).
"""

import numpy as np

import sys

for _p in ("/opt/trn_rl_repo",):
    if _p not in sys.path:
        sys.path.insert(0, _p)

import concourse.bass as bass
import concourse.mybir as mybir
import concourse.tile as tile
from concourse import bacc
from concourse import bass_utils
from concourse.bass import ts

F32 = mybir.dt.float32
F32R = mybir.dt.float32r
AF = mybir.ActivationFunctionType
ALU = mybir.AluOpType

B, N, H = 4, 1024, 64
EPS = 0.01
NB = N // 128           # 8 position blocks
N_OFF = 14              # off-diagonal block tasks per core (28 per batch / 2)
N_DIAG = 4              # diagonal block tasks per core (8 per batch / 2)
NTASK = N_OFF + N_DIAG  # 18
NCOL = NTASK * 128      # 2304 pair columns per core (128 pairs each)
OFF_COL = N_OFF * 128   # 1792 off-diagonal columns
NPOLY = 8               # polynomial degree
P_PAIRS = N * (N - 1) // 2

# Horner chunks: (col_start, col_end, engine).  Chunks never cross the
# off/diag boundary at OFF_COL; DVE/Pool widths balance their rates.
CHUNKS = [
    (0, 600, "v"),
    (600, 1200, "v"),
    (1200, OFF_COL, "v"),      # 592 cols
    (OFF_COL, NCOL, "v"),      # 512 diag cols
]

_CACHE = {}


def _build_nc():
    nc = bacc.Bacc(
        "TRN2", target_bir_lowering=False, debug=False, enable_asserts=False,
        num_devices=8,
    )

    d_lhsT = nc.dram_tensor("d_lhsT", [5, NCOL], F32, kind="ExternalInput")
    d_rhs = nc.dram_tensor("d_rhs", [5, NCOL], F32, kind="ExternalInput")
    d_coef = nc.dram_tensor("d_coef", [128, NPOLY], F32, kind="ExternalInput")
    acc_out = nc.dram_tensor("acc_out", [128, len(CHUNKS)], F32, kind="ExternalOutput")

    with tile.TileContext(nc) as tc:
        with (
            tc.tile_pool(name="consts", bufs=1) as cpool,
            tc.tile_pool(name="ps", bufs=3, space="PSUM") as pspool,
        ):
            t_lhsT = cpool.tile([128, NCOL], F32)
            t_rhs = cpool.tile([128, NCOL], F32)
            t_coef = cpool.tile([128, NPOLY], F32)
            t_y = cpool.tile([128, NCOL], F32)
            t_acc = cpool.tile([128, len(CHUNKS)], F32)
            zs = [
                (
                    cpool.tile([128, c1 - c0], F32, name=f"z{ci}a"),
                    cpool.tile([128, c1 - c0], F32, name=f"z{ci}b"),
                )
                for ci, (c0, c1, _e) in enumerate(CHUNKS)
            ]
            nc.sync.dma_start(t_lhsT[0:5, :], d_lhsT[:])
            nc.sync.dma_start(t_rhs[0:5, :], d_rhs[:])
            nc.sync.dma_start(t_coef[:], d_coef[:])

            # Phase A: y = sqrt((d^2 + eps^2)/D^2) per pair, 4 blocks per bank
            for g0 in range(0, NTASK, 4):
                g1 = min(g0 + 4, NTASK)
                ps = pspool.tile([128, 512], F32)
                for t in range(g0, g1):
                    nc.tensor.matmul(
                        ps[:, ts(t - g0, 128)],
                        t_lhsT[0:5, ts(t, 128)],
                        t_rhs[0:5, ts(t, 128)],
                        start=True, stop=True,
                    )
                nc.scalar.activation(
                    t_y[:, g0 * 128 : g1 * 128], ps[:, 0 : (g1 - g0) * 128],
                    AF.Sqrt, bias=0.0, scale=1.0,
                )

            # Phase B: Horner u(y) ~= a_0 + y*(a_1 + y*(... + y*a_n)), with
            # a_0 applied on the host.  Last step fuses the column reduction.
            for ci, (c0, c1, eng_name) in enumerate(CHUNKS):
                eng = nc.vector if eng_name == "v" else nc.gpsimd
                y = t_y[:, c0:c1]
                z0, z1 = zs[ci]
                eng.scalar_tensor_tensor(
                    z0[:], y, t_coef[:, NPOLY - 1 : NPOLY], y, ALU.mult, ALU.bypass,
                )
                cur, nxt = z0, z1
                for k in range(NPOLY - 1, 1, -1):
                    eng.scalar_tensor_tensor(
                        nxt[:], cur[:], t_coef[:, k - 1 : k], y, ALU.add, ALU.mult,
                    )
                    cur, nxt = nxt, cur
                eng.scalar_tensor_tensor(
                    nxt[:], cur[:], t_coef[:, 0:1], y, ALU.add, ALU.mult,
                    accum_out=t_acc[:, ci : ci + 1],
                )

            nc.sync.dma_start(acc_out[:], t_acc[:])

    nc.compile()
    return nc


def _core_tasks(core):
    pairs_off = [(i, j) for i in range(NB) for j in range(i + 1, NB)]
    h = core % 2
    off = pairs_off[h * N_OFF : (h + 1) * N_OFF]
    diag = [(i, i) for i in range(h * N_DIAG, (h + 1) * N_DIAG)]
    return off + diag


def _silu64(x):
    return x / (1.0 + np.exp(-x))


def _fit_poly(pos, W1, b1, W2, b2, W3, b3):
    """Chebyshev fit of u(D*y) on y in [0,1]; returns (D, a[0..NPOLY])."""
    W1d, b1d, W2d, b2d, W3d, b3d = (
        a.astype(np.float64) for a in (W1, b1, W2, b2, W3, b3)
    )
    maxnorm2 = (pos.astype(np.float64) ** 2).sum(-1).max()
    D = float(np.sqrt(4.0 * maxnorm2 + EPS * EPS))
    k = np.arange(2001)
    ynodes = 0.5 * (1.0 + np.cos(np.pi * k / 2000))
    d = D * ynodes
    h = _silu64(d[:, None] * W1d[0] + b1d)
    h = _silu64(h @ W2d + b2d)
    f = h @ W3d[:, 0] + b3d[0]
    cf = np.polynomial.chebyshev.chebfit(2.0 * ynodes - 1.0, f, NPOLY)
    poly_t = np.polynomial.Polynomial(np.polynomial.chebyshev.cheb2poly(cf))
    poly_y = poly_t(np.polynomial.Polynomial([-1.0, 2.0]))
    a = np.zeros(NPOLY + 1, np.float64)
    a[: len(poly_y.coef)] = poly_y.coef
    return D, a


def _make_in_maps(pos, D, a):
    coef = np.broadcast_to(
        a[1:].astype(np.float32), (128, NPOLY)
    ).copy()
    inv = 1.0 / (D * D)
    in_maps = []
    for core in range(8):
        b = core // 2
        pb = pos[b].astype(np.float64)
        nrm = (pb * pb).sum(-1)
        lhsT = np.zeros((5, NCOL), np.float32)
        rhs = np.zeros((5, NCOL), np.float32)
        for t, (bi, bj) in enumerate(_core_tasks(core)):
            Pi = pb[bi * 128 : (bi + 1) * 128]
            Pj = pb[bj * 128 : (bj + 1) * 128]
            sl = slice(t * 128, (t + 1) * 128)
            lhsT[:3, sl] = (-2.0 * inv) * Pi.T
            lhsT[3, sl] = (nrm[bi * 128 : (bi + 1) * 128] + EPS * EPS) * inv
            lhsT[4, sl] = 1.0
            rhs[:3, sl] = Pj.T
            rhs[3, sl] = 1.0
            rhs[4, sl] = nrm[bj * 128 : (bj + 1) * 128] * inv
        in_maps.append({"d_lhsT": lhsT, "d_rhs": rhs, "d_coef": coef})
    return in_maps


def _postprocess(results, D, a):
    # device g(y) = u(D*y) - a_0; host restores a_0 counts and halves the
    # double-counted diagonal blocks (each contains its 128 self-pairs at
    # d = eps plus every true pair twice).
    y_eps = EPS / D
    g_eps = float(np.polynomial.polynomial.polyval(y_eps, a) - a[0])
    a0 = a[0]
    n_off_slots = N_OFF * 128 * 128
    n_diag_slots = N_DIAG * 128 * 128
    n_self = N_DIAG * 128

    U = np.zeros(B, np.float64)
    for core, res in enumerate(results):
        b = core // 2
        r = res["acc_out"].astype(np.float64)  # [128, len(CHUNKS)]
        S_off = r[:, 0:3].sum()
        S_diag = r[:, 3].sum()
        off_u = S_off + n_off_slots * a0
        diag_u = (S_diag + n_diag_slots * a0 - n_self * (g_eps + a0)) / 2.0
        U[b] += off_u + diag_u
    U = U / N
    return U.reshape(B, 1).astype(np.float32)


def _run(inputs, trace=False, **kw):
    if "nc" not in _CACHE:
        _CACHE["nc"] = _build_nc()
    nc = _CACHE["nc"]
    pos = np.asarray(inputs["pos"])
    D, a = _fit_poly(
        pos, np.asarray(inputs["W1"]), np.asarray(inputs["b1"]),
        np.asarray(inputs["W2"]), np.asarray(inputs["b2"]),
        np.asarray(inputs["W3"]), np.asarray(inputs["b3"]),
    )
    in_maps = _make_in_maps(pos, D, a)
    res = bass_utils.run_bass_kernel_spmd(
        nc, in_maps, core_ids=list(range(8)), trace=trace, **kw
    )
    out = _postprocess(res.results, D, a)
    return out, res


def kernel(pos, W1, b1, W2, b2, W3, b3):
    out, _ = _run(dict(pos=pos, W1=W1, b1=b1, W2=W2, b2=b2, W3=W3, b3=b3))
    return out
